# revision 16
# baseline (speedup 1.0000x reference)
"""Causal multi-head attention (B=2, H=16, S=2048, D=64, fp32) on 8 TRN2
NeuronCores.

Sharding: the 32 (B*H) head-slices are split 4 per core (pure head
parallelism, no collectives). Each core runs flash-attention-style causal
attention for its 4 heads.

Per-core kernel design:
  * Scores are computed TRANSPOSED: S^T[k, q] = (K Q^T), k on partitions,
    q on the free axis, in chunks of [128 k, 256 q]. This makes P^T = exp(S^T)
    directly usable as the moving operand of the PV matmul (contraction = k on
    partitions), so no O(S^2) transposes are ever needed.
  * Softmax runs without max-subtraction (inputs are ~N(0,1) after scaling so
    exp() is well within fp32 range). The row-sum is obtained for free by
    appending a ones-column to V: out_aug^T = [V | 1]^T @ P^T gives the
    unnormalized output in rows 0..63 and the softmax denominator in row 64.
  * Q/K/V are cast to bf16 during the load DMA; QK^T and PV run at the 1
    cycle/row TensorE rate with fp32 PSUM accumulation.
  * Two heads are processed as a pair, stacked on SBUF partitions 0-63 /
    64-127: one PE transpose moves both heads' Q (or K) tiles into the
    [d, s] layout, and the two QK matmuls occupy disjoint PE row groups so
    the hardware can overlap them.
  * The causal mask is multiplicative (0/1 bf16) applied after exp on the two
    diagonal k-tiles only; the row-sum picks up the masking automatically
    because it is computed by the PV matmul from the masked P^T.
  * exp() on ScalarE is the throughput limit; PSUM is organized as
    2 x [128, 1536] score slabs (6 banks) + 1 bank PV accumulators + 1 bank
    drain staging so ACT instructions stay wide.
"""

import numpy as np

import concourse.bass as bass
import concourse.mybir as mybir
import concourse.tile as tile
from concourse import bacc
from concourse.bass_utils import run_bass_kernel_spmd

B, H, S, D = 2, 16, 2048, 64
N_CORES = 8
HPC = (B * H) // N_CORES  # heads per core

F32 = mybir.dt.float32
BF16 = mybir.dt.bfloat16
EXP = mybir.ActivationFunctionType.Exp


def _make_identity(nc, ap):
    nc.gpsimd.memset(ap, 0.0)
    sq = ap.shape[0]
    nc.gpsimd.affine_select(
        out=ap,
        in_=ap,
        compare_op=mybir.AluOpType.not_equal,
        fill=1.0,
        base=0,
        pattern=[[-1, sq]],
        channel_multiplier=1,
    )


def build(is_causal: bool, scale: float, s: int = S, hpc: int = HPC):
    """Build the single-core Bass graph (SPMD across cores via input shards)."""
    QC = 256           # queries per chunk (2 q-tiles of 128)
    KT = 128           # keys per tile
    n_qt = s // 128    # s-tiles per head
    n_qc = s // QC     # q-chunks per head
    SUP = 3            # k-tiles per ACT superchunk (3 PSUM banks)

    nc = bacc.Bacc("TRN2", target_bir_lowering=False)
    Qd = nc.declare_dram_parameter("Q", [hpc, s, D], F32, isOutput=False)
    Kd = nc.declare_dram_parameter("K", [hpc, s, D], F32, isOutput=False)
    Vd = nc.declare_dram_parameter("V", [hpc, s, D], F32, isOutput=False)
    Od = nc.declare_dram_parameter("out", [hpc, s, D], F32, isOutput=True)

    with TileCtx(nc) as tc:
        _build_body(nc, tc, Qd, Kd, Vd, Od, is_causal, scale, s, hpc,
                    QC, KT, n_qt, n_qc, SUP)
    nc.compile()
    return nc


class TileCtx:
    """Thin wrapper so build() reads linearly."""

    def __init__(self, nc):
        self.tc = tile.TileContext(nc)

    def __enter__(self):
        return self.tc.__enter__()

    def __exit__(self, *a):
        return self.tc.__exit__(*a)


def _build_body(nc, tc, Qd, Kd, Vd, Od, is_causal, scale, s, hpc,
                QC, KT, n_qt, n_qc, SUP):
    from contextlib import ExitStack

    with ExitStack() as ctx:
        singles = ctx.enter_context(tc.tile_pool(name="singles", bufs=1))
        nat = ctx.enter_context(tc.tile_pool(name="nat", bufs=2))
        qkt = ctx.enter_context(tc.tile_pool(name="qkt", bufs=4))
        vpool = ctx.enter_context(tc.tile_pool(name="vpool", bufs=4))
        ppool = ctx.enter_context(tc.tile_pool(name="ppool", bufs=3))
        ovpool = ctx.enter_context(tc.tile_pool(name="ovpool", bufs=2))
        recpool = ctx.enter_context(tc.tile_pool(name="recpool", bufs=4))
        outpool = ctx.enter_context(tc.tile_pool(name="outpool", bufs=6))
        spool = ctx.enter_context(tc.tile_pool(name="spool", bufs=2, space="PSUM"))
        pvpool = ctx.enter_context(tc.tile_pool(name="pvpool", bufs=2, space="PSUM"))

        # V tiles (with ones column for softmax row-sums) are allocated and
        # memset FIRST on gpsimd, so the identity build below gets later
        # Pool ticks: the prep-transpose matmuls' wait on the identity then
        # transitively covers the V memsets, keeping every Ldweights at a
        # single sync wait (hardware limit).
        vtiles = []
        for pair in range(hpc // 2):
            va = vpool.tile([128, n_qt, 65], BF16, tag="v")
            vb = vpool.tile([128, n_qt, 65], BF16, tag="v")
            nc.gpsimd.memset(va[:, :, 64:65], 1.0)
            nc.gpsimd.memset(vb[:, :, 64:65], 1.0)
            for h, vt in ((2 * pair, va), (2 * pair + 1, vb)):
                nc.gpsimd.dma_start(
                    out=vt[:, :, 0:64],
                    in_=Vd[h].rearrange("(t p) d -> p t d", p=128))
            vtiles.append((va, vb))

        ident_b = singles.tile([128, 128], BF16)
        _make_identity(nc, ident_b)
        ident_f = singles.tile([128, 128], F32)
        _make_identity(nc, ident_f)

        if is_causal:
            # mask_wide[k, u] = 1.0 if u - 128 - k >= 0 else 0.0
            # diagonal k-tile with offset o in {0, 128} uses slice
            # mask_wide[:, 128-o : 128-o+QC]  ->  m[k, q] = (q >= k + o)
            mask = singles.tile([128, 128 + QC], BF16)
            nc.gpsimd.memset(mask, 1.0)
            nc.gpsimd.affine_select(
                out=mask,
                in_=mask,
                compare_op=mybir.AluOpType.is_ge,
                fill=0.0,
                base=-128,
                pattern=[[1, 128 + QC]],
                channel_multiplier=-1,
            )

        for pair in range(hpc // 2):
            hA, hB = 2 * pair, 2 * pair + 1

            # ---- load Q/K natural layout (bf16 cast during DMA) ----
            # One contiguous tile + one DMA per head so every transpose
            # Ldweights carries a single sync wait and a 2D weight AP.
            qna = nat.tile([128, n_qt, 64], BF16, tag="nat")
            qnb = nat.tile([128, n_qt, 64], BF16, tag="nat")
            kna = nat.tile([128, n_qt, 64], BF16, tag="nat")
            knb = nat.tile([128, n_qt, 64], BF16, tag="nat")
            for h, t_ in ((hA, qna), (hB, qnb)):
                nc.gpsimd.dma_start(
                    out=t_, in_=Qd[h].rearrange("(t p) d -> p t d", p=128))
            for h, t_ in ((hA, kna), (hB, knb)):
                nc.gpsimd.dma_start(
                    out=t_, in_=Kd[h].rearrange("(t p) d -> p t d", p=128))

            va, vb = vtiles[pair]

            # ---- transpose Q/K into [d, s] layout, all on partitions
            #      0-63 (concurrent matmuls on disjoint PE row groups hang
            #      this silicon, so everything stays in row group 0-1) ----
            qt = qkt.tile([64, 2 * s], BF16, tag="qkt")
            kts = qkt.tile([64, 2 * s], BF16, tag="qkt")
            for hh, srcs in ((0, (qna, kna)), (1, (qnb, knb))):
                for src, dst in zip(srcs, (qt, kts)):
                    t = 0
                    while t < n_qt:
                        n = min(12, n_qt - t)
                        st = spool.tile([128, 1536], BF16, tag="spsum")
                        for i in range(n):
                            nc.tensor.transpose(
                                st[0:64, 128 * i:128 * (i + 1)],
                                src[:, t + i, :], ident_b)
                        nc.vector.tensor_copy(
                            dst[0:64, hh * s + 128 * t:hh * s + 128 * (t + n)],
                            st[0:64, :128 * n])
                        t += n

            # ---- main attention loop over q-chunks ----
            for qc in range(n_qc):
                q0 = QC * qc
                n_kt = (q0 + QC) // KT if is_causal else n_qt

                pv_a = pvpool.tile([128, 256], F32, tag="pvst")
                pv_b = pvpool.tile([128, 256], F32, tag="pvst")
                done = 0
                while done < n_kt:
                    nk = min(SUP, n_kt - done)
                    st = spool.tile([128, 1536], F32, tag="spsum")
                    # QK^T (transposed scores), K=64 matmuls in row group 0-1
                    for i in range(nk):
                        kti = done + i
                        for hh, coff in ((0, 0), (1, 256)):
                            nc.tensor.matmul(
                                st[:, 512 * i + coff:512 * i + coff + 256],
                                lhsT=kts[0:64, hh * s + KT * kti:
                                         hh * s + KT * (kti + 1)],
                                rhs=qt[0:64, hh * s + q0:hh * s + q0 + QC],
                                start=True, stop=True)
                    # exp over the whole superchunk in one ACT instruction
                    pt = ppool.tile([128, 1536], BF16, tag="pt")
                    nc.scalar.activation(
                        pt[:, :512 * nk], st[:, :512 * nk], EXP, scale=scale)
                    # causal mask (multiplicative) on the diagonal k-tiles
                    if is_causal:
                        for i in range(nk):
                            kti = done + i
                            if kti == 2 * qc:
                                o = 0
                            elif kti == 2 * qc + 1:
                                o = 128
                            else:
                                continue
                            for coff in (0, 256):
                                sl = pt[:, 512 * i + coff:512 * i + coff + 256]
                                nc.vector.tensor_mul(
                                    sl, sl, mask[:, 128 - o:128 - o + QC])
                    # PV accumulation (plus row-sums via the ones column)
                    for i in range(nk):
                        kti = done + i
                        for vt, coff, pvt in ((va, 0, pv_a), (vb, 256, pv_b)):
                            nc.tensor.matmul(
                                pvt[0:65, 0:256],
                                lhsT=vt[:, kti, :],
                                rhs=pt[:, 512 * i + coff:512 * i + coff + 256],
                                start=(kti == 0), stop=(kti == n_kt - 1))
                    done += nk

                # ---- drain: transpose back to [q, d], normalize, store ----
                ov = ovpool.tile([128, 512], F32, tag="ov")
                nc.vector.tensor_copy(ov[0:65, 0:256], pv_a[0:65, :])
                nc.vector.tensor_copy(ov[0:65, 256:512], pv_b[0:65, :])
                stg = spool.tile([128, 1536], F32, tag="spsum")
                for j in range(4):
                    nc.tensor.transpose(
                        stg[:, 65 * j:65 * j + 65],
                        ov[0:65, 128 * j:128 * (j + 1)],
                        ident_f[0:65, 0:65])
                for j in range(4):
                    rec = recpool.tile([128, 1], F32, tag="rec")
                    nc.vector.reciprocal(rec, stg[:, 65 * j + 64:65 * j + 65])
                    ot = outpool.tile([128, 64], F32, tag="ot")
                    nc.vector.tensor_scalar_mul(
                        ot, stg[:, 65 * j:65 * j + 64], rec)
                    h = hA if j < 2 else hB
                    qoff = q0 + 128 * (j % 2)
                    nc.sync.dma_start(out=Od[h, qoff:qoff + 128, :], in_=ot)


def shard_inputs(Q, K, V, s=S, hpc=HPC, n_cores=N_CORES):
    QH = np.ascontiguousarray(np.asarray(Q, np.float32).reshape(-1, s, D))
    KH = np.ascontiguousarray(np.asarray(K, np.float32).reshape(-1, s, D))
    VH = np.ascontiguousarray(np.asarray(V, np.float32).reshape(-1, s, D))
    in_maps = []
    for c in range(n_cores):
        sl = slice(c * hpc, (c + 1) * hpc)
        in_maps.append({
            "Q": np.ascontiguousarray(QH[sl]),
            "K": np.ascontiguousarray(KH[sl]),
            "V": np.ascontiguousarray(VH[sl]),
        })
    return in_maps


def kernel(**inputs) -> np.ndarray:
    Q = np.asarray(inputs["Q"], np.float32)
    K = np.asarray(inputs["K"], np.float32)
    V = np.asarray(inputs["V"], np.float32)
    is_causal = bool(int(np.asarray(inputs["is_causal"])))
    scale = float(np.asarray(inputs["softmax_scale"]))

    in_maps = shard_inputs(Q, K, V)
    nc = build(is_causal, scale)
    res = run_bass_kernel_spmd(nc, in_maps, core_ids=list(range(N_CORES)))
    outs = [res.results[c]["out"] for c in range(N_CORES)]
    return np.concatenate(outs, axis=0).reshape(B, H, S, D).astype(np.float32)


# revision 18
# speedup vs baseline: 1.1095x; 1.1095x over previous
"""Causal multi-head attention (B=2, H=16, S=2048, D=64, fp32) on 8 TRN2
NeuronCores.

Sharding: the 32 (B*H) head-slices are split 4 per core (pure head
parallelism, no collectives). Each core runs flash-attention-style causal
attention for its 4 heads.

Per-core kernel design (v4):
  * Scores are computed TRANSPOSED: S^T[k, q] = K Q^T, k on partitions, q on
    the free axis, in [128 k, 512 q] tiles, so P^T = exp(S^T) feeds the PV
    matmul directly (contraction = k on partitions) with no O(S^2)
    transposes.
  * ALL matmuls are zero-padded to full-array 128x128xN shape: partial-array
    matmuls (K=64 or M=65) keep the PE activity monitor below its threshold
    and the clock stays at 1.2 GHz; full-array matmuls warm it to 2.4 GHz.
    Zero rows/columns are free (array time is N cycles regardless of K/M).
  * Q/K are transposed on-chip from natural layout with zero columns
    interleaved, so each PE transpose emits the zero-padded [d|0, s] layout
    directly.
  * Softmax runs without max-subtraction; the denominator comes from a ones
    column inside the zero-padded V (row 64 of the PV output).
  * Two 512-query chunks form a superblock that reuses each K/V weight load
    for two matmuls (Ldweights is serialized on this toolchain, ~107 ns
    each).
  * Causal masking is multiplicative (0/1 bf16) after exp on diagonal
    k-tiles only; the row-sum picks it up automatically via the PV matmul.
  * Output drain avoids the PE: DVE normalizes in transposed form
    (partition-broadcast reciprocal row), the xbar DMA transposes bf16
    [64,128] -> [128,64], and a cast-DMA writes fp32 to HBM.
"""

import numpy as np

import concourse.bass as bass
import concourse.mybir as mybir
import concourse.tile as tile
from concourse import bacc
from concourse.bass_utils import run_bass_kernel_spmd

B, H, S, D = 2, 16, 2048, 64
N_CORES = 8
HPC = (B * H) // N_CORES  # heads per core

F32 = mybir.dt.float32
BF16 = mybir.dt.bfloat16
EXP = mybir.ActivationFunctionType.Exp


def _make_identity(nc, ap):
    nc.gpsimd.memset(ap, 0.0)
    sq = ap.shape[0]
    nc.gpsimd.affine_select(
        out=ap,
        in_=ap,
        compare_op=mybir.AluOpType.not_equal,
        fill=1.0,
        base=0,
        pattern=[[-1, sq]],
        channel_multiplier=1,
    )


def build(is_causal: bool, scale: float, s: int = S, hpc: int = HPC):
    QC = 512           # queries per chunk (1 PSUM bank at fp32)
    KT = 128           # keys per tile
    n_qt = s // 128    # s-tiles per head
    n_qc = s // QC     # q-chunks per head

    nc = bacc.Bacc("TRN2", target_bir_lowering=False)
    Qd = nc.declare_dram_parameter("Q", [hpc, s, D], F32, isOutput=False)
    Kd = nc.declare_dram_parameter("K", [hpc, s, D], F32, isOutput=False)
    Vd = nc.declare_dram_parameter("V", [hpc, s, D], F32, isOutput=False)
    Od = nc.declare_dram_parameter("out", [hpc, s, D], F32, isOutput=True)

    with tile.TileContext(nc) as tc:
        _build_body(nc, tc, Qd, Kd, Vd, Od, is_causal, scale, s, hpc,
                    QC, KT, n_qt, n_qc)
    nc.compile()
    return nc


def _build_body(nc, tc, Qd, Kd, Vd, Od, is_causal, scale, s, hpc,
                QC, KT, n_qt, n_qc):
    from contextlib import ExitStack

    with ExitStack() as ctx:
        singles = ctx.enter_context(tc.tile_pool(name="singles", bufs=1))
        nat = ctx.enter_context(tc.tile_pool(name="nat", bufs=2))
        qkt = ctx.enter_context(tc.tile_pool(name="qkt", bufs=4))
        vpool = ctx.enter_context(tc.tile_pool(name="vpool", bufs=2))
        ppool = ctx.enter_context(tc.tile_pool(name="ppool", bufs=3))
        rpool = ctx.enter_context(tc.tile_pool(name="rpool", bufs=4))
        npool = ctx.enter_context(tc.tile_pool(name="npool", bufs=4))
        outpool = ctx.enter_context(tc.tile_pool(name="outpool", bufs=8))
        spool = ctx.enter_context(tc.tile_pool(name="spool", bufs=2, space="PSUM"))
        pvpool = ctx.enter_context(tc.tile_pool(name="pvpool", bufs=4, space="PSUM"))

        ident_b = singles.tile([128, 128], BF16)
        _make_identity(nc, ident_b)

        if is_causal:
            # mask_wide[k, u] = 1.0 iff u - 384 - k >= 0; for a diagonal
            # k-tile with offset o (valid iff q >= k + o, o in {0,128,256,
            # 384}) use slice mask_wide[:, 384-o : 896-o].
            mask = singles.tile([128, 384 + QC], BF16)
            nc.gpsimd.memset(mask, 1.0)
            nc.gpsimd.affine_select(
                out=mask,
                in_=mask,
                compare_op=mybir.AluOpType.is_ge,
                fill=0.0,
                base=-384,
                pattern=[[1, 384 + QC]],
                channel_multiplier=-1,
            )

        for h in range(hpc):
            # ---- natural-layout loads, zero columns interleaved so the PE
            #      transposes emit the zero-padded [d|0, s] layout ----
            qn = nat.tile([128, n_qt, 128], BF16, tag="nat")
            kn = nat.tile([128, n_qt, 128], BF16, tag="nat")
            vp = vpool.tile([128, n_qt, 128], BF16, tag="v")
            nc.gpsimd.memset(qn[:, :, 64:128], 0.0)
            nc.gpsimd.memset(kn[:, :, 64:128], 0.0)
            nc.gpsimd.memset(vp[:, :, 64:128], 0.0)
            nc.gpsimd.memset(vp[:, :, 64:65], 1.0)
            nc.gpsimd.dma_start(
                out=qn[:, :, 0:64],
                in_=Qd[h].rearrange("(t p) d -> p t d", p=128))
            nc.gpsimd.dma_start(
                out=kn[:, :, 0:64],
                in_=Kd[h].rearrange("(t p) d -> p t d", p=128))
            nc.gpsimd.dma_start(
                out=vp[:, :, 0:64],
                in_=Vd[h].rearrange("(t p) d -> p t d", p=128))

            # ---- transpose Q/K to [d|0, s] (rows 64-127 zero) ----
            qt = qkt.tile([128, s], BF16, tag="qkt")
            kt = qkt.tile([128, s], BF16, tag="qkt")
            for src, dst in ((qn, qt), (kn, kt)):
                t = 0
                while t < n_qt:
                    n = min(12, n_qt - t)
                    st = spool.tile([128, 2048], BF16, tag="spsum")
                    for i in range(n):
                        nc.tensor.transpose(
                            st[:, 128 * i:128 * (i + 1)],
                            src[:, t + i, :], ident_b)
                    nc.vector.tensor_copy(
                        dst[:, 128 * t:128 * (t + n)], st[:, :128 * n])
                    t += n

            # ---- main loop: superblocks of 2 chunks (1024 queries) ----
            for sb in range(n_qc // 2):
                q0 = 2 * QC * sb
                n_kt = (q0 + 2 * QC) // KT if is_causal else n_qt
                nc0 = (q0 + QC) // KT if is_causal else n_qt  # c0's k-tiles
                pv0 = pvpool.tile([128, 512], F32, tag="pvst")
                pv1 = pvpool.tile([128, 512], F32, tag="pvst")
                for kti in range(n_kt):
                    c0 = kti < nc0
                    off = 0 if c0 else 512
                    st = spool.tile([128, 1024], F32, tag="spsum")
                    # QK^T: one K weight load, up to two N=512 matmuls
                    if c0:
                        nc.tensor.matmul(
                            st[:, 0:512],
                            lhsT=kt[:, KT * kti:KT * (kti + 1)],
                            rhs=qt[:, q0:q0 + QC],
                            start=True, stop=True)
                    nc.tensor.matmul(
                        st[:, 512:1024],
                        lhsT=kt[:, KT * kti:KT * (kti + 1)],
                        rhs=qt[:, q0 + QC:q0 + 2 * QC],
                        start=True, stop=True)
                    pt = ppool.tile([128, 1024], BF16, tag="pt")
                    nc.scalar.activation(
                        pt[:, off:1024], st[:, off:1024], EXP, scale=scale)
                    if is_causal:
                        for ci in (0, 1):
                            if ci == 0 and not c0:
                                continue
                            o = KT * kti - (q0 + QC * ci)
                            if 0 <= o < QC:
                                sl = pt[:, 512 * ci:512 * ci + QC]
                                nc.vector.tensor_mul(
                                    sl, sl, mask[:, 384 - o:384 - o + QC])
                    # PV: one V weight load, up to two matmuls; ones column
                    # in V row 64 accumulates the softmax denominators
                    if c0:
                        nc.tensor.matmul(
                            pv0[:, :],
                            lhsT=vp[:, kti, :],
                            rhs=pt[:, 0:512],
                            start=(kti == 0), stop=(kti == nc0 - 1))
                    nc.tensor.matmul(
                        pv1[:, :],
                        lhsT=vp[:, kti, :],
                        rhs=pt[:, 512:1024],
                        start=(kti == 0), stop=(kti == n_kt - 1))

                # ---- drain: bf16 copy of [out^T | rowsum | 0-pad] rows,
                #      xbar DMA-transpose to [q, d] layout, then normalize
                #      with a per-partition reciprocal (no PE work) ----
                for ci, pv in enumerate((pv0, pv1)):
                    qc0 = q0 + QC * ci
                    ov = npool.tile([80, 512], BF16, tag="ov")
                    nc.vector.tensor_copy(ov, pv[0:80, :])
                    for j in range(4):
                        tt = outpool.tile([128, 80], BF16, tag="tt")
                        nc.sync.dma_start_transpose(
                            tt, ov[:, 128 * j:128 * (j + 1)])
                        rec = rpool.tile([128, 1], F32, tag="rec")
                        nc.vector.reciprocal(rec, tt[:, 64:65])
                        ot = outpool.tile([128, 64], F32, tag="ot")
                        nc.vector.tensor_scalar_mul(ot, tt[:, 0:64], rec)
                        nc.sync.dma_start(
                            out=Od[h, qc0 + 128 * j:qc0 + 128 * (j + 1), :],
                            in_=ot)


def shard_inputs(Q, K, V, s=S, hpc=HPC, n_cores=N_CORES):
    QH = np.ascontiguousarray(np.asarray(Q, np.float32).reshape(-1, s, D))
    KH = np.ascontiguousarray(np.asarray(K, np.float32).reshape(-1, s, D))
    VH = np.ascontiguousarray(np.asarray(V, np.float32).reshape(-1, s, D))
    in_maps = []
    for c in range(n_cores):
        sl = slice(c * hpc, (c + 1) * hpc)
        in_maps.append({
            "Q": np.ascontiguousarray(QH[sl]),
            "K": np.ascontiguousarray(KH[sl]),
            "V": np.ascontiguousarray(VH[sl]),
        })
    return in_maps


def kernel(**inputs) -> np.ndarray:
    Q = np.asarray(inputs["Q"], np.float32)
    K = np.asarray(inputs["K"], np.float32)
    V = np.asarray(inputs["V"], np.float32)
    is_causal = bool(int(np.asarray(inputs["is_causal"])))
    scale = float(np.asarray(inputs["softmax_scale"]))

    in_maps = shard_inputs(Q, K, V)
    nc = build(is_causal, scale)
    res = run_bass_kernel_spmd(nc, in_maps, core_ids=list(range(N_CORES)))
    outs = [res.results[c]["out"] for c in range(N_CORES)]
    return np.concatenate(outs, axis=0).reshape(B, H, S, D).astype(np.float32)
